# revision 12
# baseline (speedup 1.0000x reference)
"""Multi-head attention (B=2, L=4096, C=512, H=8, Dh=64) on 8 trn2 cores.

Sharding: data-parallel over batch (4 cores per batch element) x
tensor-parallel over heads (2 heads per core). Each core computes a partial
output projection; the host sums the 4 partials per batch element and adds
the bias.

Per-core kernel (all matmuls in float32r; scores never hit HBM):
  - inputs: xT [512, 4096] (= x[b].T), wq/wk/wv [512, 128] col slices
    (1/sqrt(Dh) folded into wq), wo [128, 512] row slice
  - Q^T, K^T [128, 4096] via lhsT=W-chunk, rhs=xT          (transposed layout)
  - V [4096, 128] via lhsT=xT-chunk, rhs=wv, stored as [128, 32, 129] with a
    shared ones column at index 64 (head0 uses cols 0:65, head1 cols 64:129)
  - per head, per q-chunk of 1024: S^T[k,q] tiles [128, 1024] in PSUM
    (2 matmuls, K=64), one ACT exp -> P^T in SBUF, 2 AV matmuls accumulating
    att^T [65, 1024] in PSUM over 32 k-tiles; the ones column makes
    partition 64 (head0) / 0 (head1) the softmax denominator
  - normalize via reciprocal + partition_broadcast + DVE mult
  - out-proj: out[q,:] = att^T.T @ wo, one matmul per 128-row q-tile
"""

import numpy as np

B, L, C, H = 2, 4096, 512, 8
DH = C // H  # 64
P = 128
NCORES = 8
HEADS_PER_CORE = 2
CORES_PER_BATCH = 4

QCHUNK = 1024  # q columns per attention block (2 PSUM banks)
NQC = L // QCHUNK  # 4
NKT = L // P  # 32 k-tiles
NCC = C // P  # 4 contraction chunks for projections

_cached = {}


def _build():
    import concourse.mybir as mybir
    import concourse.tile as tile
    from concourse import bacc

    F32R = mybir.dt.float32r
    F32 = mybir.dt.float32
    EXP = mybir.ActivationFunctionType.Exp
    MULT = mybir.AluOpType.mult

    nc = bacc.Bacc("TRN2", target_bir_lowering=False, debug=False,
                   num_devices=NCORES)
    xT = nc.dram_tensor("xT", [C, L], F32R, kind="ExternalInput").ap()
    wq = nc.dram_tensor("wq", [C, P], F32R, kind="ExternalInput").ap()
    wk = nc.dram_tensor("wk", [C, P], F32R, kind="ExternalInput").ap()
    wv = nc.dram_tensor("wv", [C, P], F32R, kind="ExternalInput").ap()
    wo = nc.dram_tensor("wo", [P, C], F32R, kind="ExternalInput").ap()
    out = nc.dram_tensor("out", [L, C], F32R, kind="ExternalOutput").ap()

    with tile.TileContext(nc) as tc:
        with (
            tc.tile_pool(name="persist", bufs=1) as persist,
            tc.tile_pool(name="xpool", bufs=1) as xpool,
            tc.tile_pool(name="ptp", bufs=3) as ptp,
            tc.tile_pool(name="small", bufs=2) as small,
            tc.tile_pool(name="outp", bufs=3) as outp,
        ):
            # ---- load inputs ----
            wq_t = persist.tile([P, NCC, P], F32R)
            wk_t = persist.tile([P, NCC, P], F32R)
            wv_t = persist.tile([P, NCC, P], F32R)
            wo_t = persist.tile([P, C], F32R)
            nc.sync.dma_start(wq_t, wq.rearrange("(k p) m -> p k m", p=P))
            nc.sync.dma_start(wk_t, wk.rearrange("(k p) m -> p k m", p=P))
            nc.sync.dma_start(wv_t, wv.rearrange("(k p) m -> p k m", p=P))
            nc.sync.dma_start(wo_t, wo)

            xt = xpool.tile([P, NCC, L], F32R)
            xTr = xT.rearrange("(k p) n -> p k n", p=P)
            for j in range(8):  # split the 8.4MB load across DMA queues
                sl = slice(j * (L // 8), (j + 1) * (L // 8))
                nc.sync.dma_start(xt[:, :, sl], xTr[:, :, sl])

            qT = persist.tile([P, L], F32R)
            kT = persist.tile([P, L], F32R)
            # per head: [V_h (64) | ones (1)] -> 65 cols each, 130 total
            v_store = persist.tile([P, NKT, 2 * (DH + 1)], F32R)
            attn = persist.tile([P, L], F32R)

            # ---- projections ----
            with tc.tile_pool(name="pj_ps", bufs=2, space="PSUM") as pj_ps:
                # Q^T / K^T: [128 (2 heads x 64), L]
                for dst, w_t in ((qT, wq_t), (kT, wk_t)):
                    for j in range(L // 512):
                        ps = pj_ps.tile([P, 512], F32, tag="qk_ps")
                        for c in range(NCC):
                            nc.tensor.matmul(
                                ps, w_t[:, c, :],
                                xt[:, c, j * 512:(j + 1) * 512],
                                start=(c == 0), stop=(c == NCC - 1),
                            )
                        nc.vector.tensor_copy(
                            dst[:, j * 512:(j + 1) * 512], ps)

                # V: per 128-token tile, [tokens, 128] = xT-chunk.T @ wv
                ones_t = small.tile([P, NKT], F32, tag="ones")
                nc.vector.memset(ones_t, 1.0)
                nc.vector.tensor_copy(v_store[:, :, DH], ones_t)
                nc.vector.tensor_copy(v_store[:, :, 2 * DH + 1], ones_t)
                for r in range(NKT):
                    ps = pj_ps.tile([P, P], F32, tag="v_ps")
                    for c in range(NCC):
                        nc.tensor.matmul(
                            ps, xt[:, c, r * P:(r + 1) * P], wv_t[:, c, :],
                            start=(c == 0), stop=(c == NCC - 1),
                        )
                    nc.vector.tensor_copy(v_store[:, r, 0:DH], ps[:, 0:DH])
                    nc.vector.tensor_copy(
                        v_store[:, r, DH + 1:2 * DH + 1], ps[:, DH:2 * DH])

            # ---- attention ----
            s_ps_cm = tc.tile_pool(name="s_ps", bufs=2, space="PSUM")
            a_ps_cm = tc.tile_pool(name="a_ps", bufs=2, space="PSUM")
            s_ps = s_ps_cm.__enter__()
            a_ps = a_ps_cm.__enter__()
            for h in range(HEADS_PER_CORE):
                hsl = slice(h * DH, (h + 1) * DH)
                vsl = slice(h * (DH + 1), (h + 1) * (DH + 1))
                denom_row = DH
                att_lo = 0
                for qc in range(NQC):
                    qsl = slice(qc * QCHUNK, (qc + 1) * QCHUNK)
                    att = a_ps.tile([DH + 1, QCHUNK], F32, tag="att")
                    for kt in range(NKT):
                        sps = s_ps.tile([P, QCHUNK], F32, tag="spsum")
                        for j in range(QCHUNK // 512):
                            nc.tensor.matmul(
                                sps[:, j * 512:(j + 1) * 512],
                                kT[hsl, kt * P:(kt + 1) * P],
                                qT[hsl, qc * QCHUNK + j * 512:
                                   qc * QCHUNK + (j + 1) * 512],
                                start=True, stop=True,
                            )
                        pt = ptp.tile([P, QCHUNK], F32R, tag="pt")
                        nc.scalar.activation(pt, sps, EXP)
                        for j in range(QCHUNK // 512):
                            nc.tensor.matmul(
                                att[:, j * 512:(j + 1) * 512],
                                v_store[:, kt, vsl],
                                pt[:, j * 512:(j + 1) * 512],
                                start=(kt == 0), stop=(kt == NKT - 1),
                            )
                    recip = small.tile([1, QCHUNK], F32, tag="recip")
                    nc.vector.reciprocal(
                        recip, att[denom_row:denom_row + 1, :])
                    rb = small.tile([DH, QCHUNK], F32, tag="rb")
                    nc.gpsimd.partition_broadcast(rb, recip)
                    nc.vector.tensor_tensor(
                        attn[hsl, qsl], att[att_lo:att_lo + DH, :], rb, MULT)

            a_ps_cm.__exit__(None, None, None)
            s_ps_cm.__exit__(None, None, None)

            # ---- output projection ----
            with tc.tile_pool(name="o_ps", bufs=3, space="PSUM") as o_ps:
                for qt in range(L // P):
                    ps = o_ps.tile([P, C], F32, tag="o_ps")
                    nc.tensor.matmul(ps, attn[:, qt * P:(qt + 1) * P], wo_t,
                                     start=True, stop=True)
                    osb = outp.tile([P, C], F32R, tag="osb")
                    nc.vector.tensor_copy(osb, ps)
                    nc.sync.dma_start(out[qt * P:(qt + 1) * P, :], osb)

    nc.compile()
    return nc


def _get_nc():
    if "nc" not in _cached:
        _cached["nc"] = _build()
    return _cached["nc"]


def _build_in_maps(inputs):
    x = np.asarray(inputs["x"], dtype=np.float32)
    Wq = np.asarray(inputs["Wq"], dtype=np.float32)
    Wk = np.asarray(inputs["Wk"], dtype=np.float32)
    Wv = np.asarray(inputs["Wv"], dtype=np.float32)
    Wo = np.asarray(inputs["Wo"], dtype=np.float32)

    scale = np.float32(1.0 / np.sqrt(DH))
    in_maps = []
    for core in range(NCORES):
        b = core // CORES_PER_BATCH
        j = core % CORES_PER_BATCH
        csl = slice(j * P, (j + 1) * P)
        in_maps.append({
            "xT": np.ascontiguousarray(x[b].T),
            "wq": np.ascontiguousarray(Wq[:, csl] * scale),
            "wk": np.ascontiguousarray(Wk[:, csl]),
            "wv": np.ascontiguousarray(Wv[:, csl]),
            "wo": np.ascontiguousarray(Wo[csl, :]),
        })
    return in_maps


def kernel(x, Wq, Wk, Wv, Wo, bo):
    from concourse import bass_utils

    bo = np.asarray(bo, dtype=np.float32)
    in_maps = _build_in_maps(
        {"x": x, "Wq": Wq, "Wk": Wk, "Wv": Wv, "Wo": Wo})

    res = bass_utils.run_bass_kernel_spmd(
        _get_nc(), in_maps, core_ids=list(range(NCORES)))

    out = np.zeros((B, L, C), dtype=np.float32)
    for core in range(NCORES):
        out[core // CORES_PER_BATCH] += res.results[core]["out"]
    out += bo[None, None, :]
    return out
